# revision 28
# baseline (speedup 1.0000x reference)
"""DigitCaps dynamic-routing kernel for 8 TRN2 NeuronCores.

Strategy: shard the C=1152 input capsules across the 8 cores (144 each) and
keep the full batch B=256 on every core.  The routing iterations use the
factored form (never materializing u_hat = x @ W, which would be 189 MB):

  s[b,u,o]    = sum_{c,i} x[b,i,c] * (coef[c,u] * W[c,u,o,i])     (matmul, K=(c,i))
  v           = squash(s)
  G[ci,uo]    = sum_b x[b,i,c] * v[b,u,o]                          (matmul, K=b)
  agr[ci',u]  = (1/B) * sum_{o} W*G summed over i via sel2 matmul  (replicated
                to all 128 (c,i) partitions so softmax runs there directly)
  b_logits   += agr ; coef = softmax_u(b_logits)                   (c-local)

Cross-core traffic per routing iteration: one fp16 AllReduce of the partial
s ([256,160], 80KB) — the collective does the 8-way sum, so no gather DMAs or
local tree-sum.  The final iteration uses an AllToAll instead (a pure
permutation — fewer mesh rounds than a ReduceScatter's reduce tree, measured
9.3us vs 12.7us): each core receives all 8 partials for its 32 batch rows,
sums them locally in fp32, squashes, and returns a sharded output that the
host concatenates.

All matmul operands are bf16 (x, W, W_eff, v); accumulation stays fp32 in
PSUM.  Measured output error ~3.8e-3 vs the fp32 reference (tolerance 2e-2).
Iteration 0's uniform coef=0.1 is applied as a 0.1 scale in the PSUM->wire
copy of s.  The squash chain runs on the vector engine (gpsimd's semaphore
wake latency outweighs its parallelism and it cannot touch PSUM).  The
agreement -> logits -> softmax -> W_eff work is pipelined per 3-tile k-block:
as each G block closes, its W*G multiply, replicated i-sum matmul, exp and
W_eff chunk run immediately, and the next iteration's s-matmuls chase the
W_eff chunks.  PSUM note: a matmul with start=True zeroes its whole 2KB bank,
so accumulation groups sharing a bank must be strictly sequential (m-outer /
t-inner G loop; an interleaved two-bank variant of the s-matmul measured
slower and was reverted).

The collective DRAM tiles MUST stay double-buffered (dram pool bufs=2):
a bufs=1 variant serialized the iterations (+90us median) — the rotation is
load-bearing, not overhead.

Rejected alternatives, for future reference: an fp8-e4m3 collective wire
fails the accuracy gate (2.4e-2 worst-case vs 2e-2); a custom SBUF->SBUF
collective via remote_dma_broadcast with XOR-relative slots is blocked by
the Tile scheduler's deadlock check, which cannot model remotely-incremented
semaphores — it would need raw-bass (non-Tile) sections.

Current balance (iteration period ~30.6us = ~16.5 compute + ~14 collective):
the vector engine is the saturated resource in the compute block (~8.3us
back-to-back: pm muls, segmented reduces, softmax slices, W_eff chunks).
Offloads are blocked by engine constraints — gpsimd cannot touch PSUM or do
X-axis reduces and runs broadcast-muls ~3.4x slower than vector; reciprocal
is vector-only.  The collective window is runtime-internal (engine-side sem
wait + ~2us CC-queue spin-up + latency-bound mesh rounds).
"""

import os
import sys

# Prefer the Mesh collective algorithm (RDH measured slower at this size).
os.environ.setdefault("NEURON_RT_DBG_RDH_CC", "0")

if "/opt/trn_rl_repo" not in sys.path:
    sys.path.insert(0, "/opt/trn_rl_repo")

import numpy as np
import ml_dtypes

import concourse.bacc as bacc
import concourse.tile as tile
from concourse import mybir
from concourse.bass_utils import run_bass_kernel_spmd

F32 = mybir.dt.float32
F16 = mybir.dt.float16
BF16 = mybir.dt.bfloat16

B = 256          # batch
IU = 8           # in_unit (i)
C = 1152         # input capsules
U = 10           # output capsules
O = 16           # unit size
N_CORES = 8
CL = C // N_CORES          # 144 local capsules
CI = CL * IU               # 1152 local (c,i) rows
K = CI // 128              # 9 contraction tiles
UO = U * O                 # 160
BL = B // N_CORES          # 32 batch rows per core in the scattered output
NROUTE = 4


def _build_program():
    nc = bacc.Bacc(
        "TRN2",
        target_bir_lowering=False,
        debug=False,
        enable_asserts=False,
        num_devices=N_CORES,
    )

    xp_d = nc.dram_tensor("xp", [128, K * B], BF16, kind="ExternalInput").ap()
    xb_d = nc.dram_tensor("xb", [128, 2 * CI], BF16, kind="ExternalInput").ap()
    w1_d = nc.dram_tensor("w1", [128, K * UO], BF16, kind="ExternalInput").ap()
    sel2_d = nc.dram_tensor("sel2", [128, 128], BF16, kind="ExternalInput").ap()
    out_d = nc.dram_tensor("out", [BL, U, O, 1], F32, kind="ExternalOutput").ap()

    with tile.TileContext(nc) as tc:
        with (
            tc.tile_pool(name="persist", bufs=1) as pp,
            tc.tile_pool(name="work", bufs=2) as wp,
            tc.tile_pool(name="sps", bufs=1, space="PSUM") as sps,
            tc.tile_pool(name="gps", bufs=1, space="PSUM") as gps,
            tc.tile_pool(name="aps", bufs=1, space="PSUM") as aps,
            tc.tile_pool(name="dram", bufs=2, space="DRAM") as dram,
        ):
            # ---- persistent tiles ----
            xp_s = pp.tile([128, K * B], BF16, tag="xp")
            xb_s = pp.tile([128, 2 * CI], BF16, tag="xb")
            w1_s = pp.tile([128, K * UO], BF16, tag="w1")
            weff_s = pp.tile([128, K * UO], BF16, tag="weff")
            v_s = pp.tile([128, 2 * UO], BF16, tag="v")
            sel2_s = pp.tile([128, 128], BF16, tag="sel2")
            pm2_s = pp.tile([128, K * U], BF16, tag="pm2")
            b_state = pp.tile([128, K * U], F32, tag="bstate")
            scr = pp.tile([128, 2], F32, tag="scr")   # ACT table prewarm scratch

            nc.gpsimd.memset(scr[:], 1.0)

            # ---- input loads: xp/w1 chunks interleaved on sync+scalar so the
            # k-tiles of the iter-0 s-matmul become ready in order; xb/sel2
            # queue behind them (not needed until after the first collective)
            xpk = lambda a, b: (xp_s[:, a * B:b * B], xp_d[:, a * B:b * B])
            w1k = lambda a, b: (w1_s[:, a * UO:b * UO], w1_d[:, a * UO:b * UO])
            # tiny k0 chunks first so the iter-0 s-matmul starts ~2us earlier;
            # the remaining tiles follow in pair-matched chunks on both queues
            nc.sync.dma_start(*xpk(0, 1))
            nc.scalar.dma_start(*w1k(0, 1))
            nc.sync.dma_start(*xpk(1, 3))
            nc.scalar.dma_start(*w1k(1, 3))
            nc.sync.dma_start(*w1k(3, 6))
            nc.scalar.dma_start(*xpk(3, 6))
            nc.sync.dma_start(*xpk(6, K))
            nc.scalar.dma_start(*w1k(6, K))
            nc.sync.dma_start(xb_s[:, :CI], xb_d[:, :CI])
            nc.scalar.dma_start(xb_s[:, CI:], xb_d[:, CI:])
            nc.sync.dma_start(sel2_s[:], sel2_d)

            weff = w1_s  # iteration 0: coef folded into a 0.1 scale of s

            for r in range(NROUTE):
                last = r == NROUTE - 1
                sfx = "_l" if last else ""

                # ---- s partial: [b,(u,o)] += xp^T @ weff over (c,i) tiles ----
                s_stage = wp.tile([128, 2 * UO], F16, tag="s_stage" + sfx)
                s_ps = sps.tile([128, 2 * 256], F32, tag="s_ps")
                scale0 = 0.1 if r == 0 else 1.0
                for g in range(2):
                    for k in range(K):
                        nc.tensor.matmul(
                            s_ps[:, g * 256: g * 256 + UO],
                            lhsT=xp_s[:, k * B + g * 128: k * B + (g + 1) * 128],
                            rhs=weff[:, k * UO:(k + 1) * UO],
                            start=(k == 0), stop=(k == K - 1),
                        )
                    if g == 0:
                        if r == 0:
                            nc.vector.tensor_scalar_mul(
                                s_stage[:, :UO], s_ps[:, :UO], scale0
                            )
                        else:
                            nc.vector.tensor_copy(s_stage[:, :UO], s_ps[:, :UO])
                    else:
                        nc.scalar.activation(
                            s_stage[:, UO:],
                            s_ps[:, 256: 256 + UO],
                            mybir.ActivationFunctionType.Copy,
                            scale=scale0,
                        )

                # ---- collective: AllReduce (inner) / AllToAll (last) ----
                cc_in = dram.tile([B, UO], F16, tag="cc_in" + sfx)
                cin_view = cc_in.opt().rearrange("(g p) f -> p g f", g=2)
                nc.sync.dma_start(cin_view[:, 0], s_stage[:, :UO])
                nc.scalar.dma_start(cin_view[:, 1], s_stage[:, UO:])
                if last:
                    # AllToAll (pure permutation, fewer mesh rounds than a
                    # ReduceScatter's reduce tree): each core receives all 8
                    # partials for its 32 batch rows and sums them locally.
                    cc_out = dram.tile([B, UO], F16, tag="cc_out_l")
                    nc.gpsimd.collective_compute(
                        "AllToAll",
                        mybir.AluOpType.bypass,
                        replica_groups=[list(range(N_CORES))],
                        ins=[cc_in.opt()],
                        outs=[cc_out.opt()],
                    )
                    sa = wp.tile([BL, 8 * UO], F16, tag="sa")
                    nc.sync.dma_start(
                        sa[:].rearrange("p (q f) -> p q f", q=8),
                        cc_out.opt().rearrange("(q p) f -> p q f", q=8),
                    )
                    t1l = wp.tile([BL, 4 * UO], F32, tag="t1l")
                    nc.vector.tensor_add(t1l[:], sa[:, :4 * UO], sa[:, 4 * UO:])
                    t2l = wp.tile([BL, 2 * UO], F32, tag="t2l")
                    nc.vector.tensor_add(t2l[:], t1l[:, :2 * UO], t1l[:, 2 * UO:])
                    s32 = wp.tile([BL, UO], F32, tag="s32")
                    nc.vector.tensor_add(s32[:], t2l[:, :UO], t2l[:, UO:])
                    # squash the 32-row shard and write the sharded output
                    sq32 = wp.tile([BL, UO], F32, tag="sq32")
                    nc.vector.tensor_mul(sq32[:], s32[:], s32[:])
                    n232 = wp.tile([BL, U], F32, tag="n232")
                    nc.vector.reduce_sum(
                        n232[:], sq32[:].rearrange("p (u o) -> p u o", u=U),
                        axis=mybir.AxisListType.X,
                    )
                    rt32 = wp.tile([BL, U], F32, tag="rt32")
                    nc.scalar.sqrt(rt32[:], n232[:])
                    dn32 = wp.tile([BL, U], F32, tag="dn32")
                    nc.vector.tensor_scalar_add(dn32[:], n232[:], 1.0)
                    rd32 = wp.tile([BL, U], F32, tag="rd32")
                    nc.vector.reciprocal(rd32[:], dn32[:])
                    f32t = wp.tile([BL, U], F32, tag="f32t")
                    nc.vector.tensor_mul(f32t[:], rt32[:], rd32[:])
                    vlast = wp.tile([BL, UO], F32, tag="vlast")
                    nc.vector.tensor_mul(
                        vlast[:].rearrange("p (u o) -> p u o", u=U),
                        s32[:].rearrange("p (u o) -> p u o", u=U),
                        f32t[:].unsqueeze(2).broadcast_to((BL, U, O)),
                    )
                    nc.sync.dma_start(
                        out_d.rearrange("p u o one -> p (u o one)"), vlast[:]
                    )
                    break

                cc_out = dram.tile([B, UO], F16, tag="cc_out", addr_space="Shared")
                nc.gpsimd.collective_compute(
                    "AllReduce",
                    mybir.AluOpType.add,
                    replica_groups=[list(range(N_CORES))],
                    ins=[cc_in.opt()],
                    outs=[cc_out.opt()],
                )
                s_sb = wp.tile([128, 2 * UO], F16, tag="s_sb")
                cc_view = cc_out.opt().rearrange("(g p) f -> p g f", g=2)
                nc.sync.dma_start(s_sb[:, :UO], cc_view[:, 0])
                nc.scalar.dma_start(s_sb[:, UO:], cc_view[:, 1])

                # ---- squash: v = s * sqrt(n2) / (1 + n2), split across engines ----
                sq = wp.tile([128, 2 * UO], F32, tag="sq")
                n2 = wp.tile([128, 2 * U], F32, tag="n2")
                nc.vector.tensor_mul(sq[:], s_sb[:], s_sb[:])
                nc.vector.reduce_sum(
                    n2[:], sq[:].rearrange("p (t u o) -> p (t u) o", t=2, u=U),
                    axis=mybir.AxisListType.X,
                )
                rt = wp.tile([128, 2 * U], F32, tag="rt")
                nc.scalar.sqrt(rt[:], n2[:])
                # prewarm the Exp ACT table while G runs (dep on rt orders it)
                nc.scalar.activation(
                    scr[:, 1:2], rt[:, 0:1], mybir.ActivationFunctionType.Exp
                )
                dn = wp.tile([128, 2 * U], F32, tag="dn")
                nc.vector.tensor_scalar_add(dn[:], n2[:], 1.0)
                rd = wp.tile([128, 2 * U], F32, tag="rd")
                nc.vector.reciprocal(rd[:], dn[:])
                f = wp.tile([128, 2 * U], F32, tag="f")
                nc.vector.tensor_mul(f[:], rt[:], rd[:])
                for t in range(2):
                    nc.vector.tensor_mul(
                        v_s[:, t * UO:(t + 1) * UO].rearrange("p (u o) -> p u o", u=U),
                        s_sb[:, t * UO:(t + 1) * UO].rearrange("p (u o) -> p u o", u=U),
                        f[:, t * U:(t + 1) * U].unsqueeze(2).broadcast_to((128, U, O)),
                    )

                # ---- G[(c,i),(u,o)] = sum_b x*v : t-outer so t=0 starts early ----
                g_ps = [
                    gps.tile([128, 3 * UO], F32, tag=f"g_ps{j}", name=f"g_ps{j}")
                    for j in range(3)
                ]
                for m in range(K):
                    j, mm = divmod(m, 3)
                    for t in range(2):
                        nc.tensor.matmul(
                            g_ps[j][:, mm * UO: (mm + 1) * UO],
                            lhsT=xb_s[:, t * CI + m * 128: t * CI + (m + 1) * 128],
                            rhs=v_s[:, t * UO:(t + 1) * UO],
                            start=(t == 0), stop=(t == 1),
                        )
                # ---- per-k-block pipeline: as each 3-tile G block closes, run
                # its agreement multiply, replicated i-sum, logit update,
                # softmax slice and W_eff chunk; the s-matmuls chase chunks ----
                pm = wp.tile([128, K * UO], BF16, tag="pm")
                a_ps = aps.tile([128, K * U], F32, tag="a_ps")
                eb = wp.tile([128, K * U], F32, tag="eb")
                den = wp.tile([128, K], F32, tag="den")
                rden = wp.tile([128, K], F32, tag="rden")
                cnorm = wp.tile([128, K * U], F32, tag="cnorm")
                for j in range(3):
                    sU, eU = 3 * j * U, 3 * (j + 1) * U
                    sF, eF = 3 * j * UO, 3 * (j + 1) * UO
                    nc.vector.tensor_mul(pm[:, sF:eF], g_ps[j][:], w1_s[:, sF:eF])
                    with nc.allow_low_precision(reason="bf16 agreement pm2"):
                        nc.vector.reduce_sum(
                            pm2_s[:, sU:eU].rearrange("p (m u) -> p m u", m=3),
                            pm[:, sF:eF].rearrange("p (m u o) -> p m u o", m=3, u=U),
                            axis=mybir.AxisListType.X,
                        )
                    nc.tensor.matmul(
                        a_ps[:, sU:eU], lhsT=sel2_s[:], rhs=pm2_s[:, sU:eU],
                        start=True, stop=True,
                    )
                    if r == 0:
                        nc.vector.tensor_copy(b_state[:, sU:eU], a_ps[:, sU:eU])
                    else:
                        nc.vector.tensor_add(
                            b_state[:, sU:eU], b_state[:, sU:eU], a_ps[:, sU:eU]
                        )
                    nc.scalar.activation(
                        eb[:, sU:eU], b_state[:, sU:eU],
                        mybir.ActivationFunctionType.Exp,
                    )
                    nc.vector.reduce_sum(
                        den[:, 3 * j:3 * j + 3],
                        eb[:, sU:eU].rearrange("p (k u) -> p k u", k=3),
                        axis=mybir.AxisListType.X,
                    )
                    nc.vector.reciprocal(
                        rden[:, 3 * j:3 * j + 3], den[:, 3 * j:3 * j + 3]
                    )
                    nc.vector.tensor_mul(
                        cnorm[:, sU:eU].rearrange("p (k u) -> p k u", k=3),
                        eb[:, sU:eU].rearrange("p (k u) -> p k u", k=3),
                        rden[:, 3 * j:3 * j + 3].unsqueeze(2).broadcast_to((128, 3, U)),
                    )
                    nc.vector.tensor_mul(
                        weff_s[:, sF:eF].rearrange("p (k u o) -> p k u o", k=3, u=U),
                        w1_s[:, sF:eF].rearrange("p (k u o) -> p k u o", k=3, u=U),
                        cnorm[:, sU:eU].rearrange("p (k u) -> p k u", k=3)
                        .unsqueeze(3).broadcast_to((128, 3, U, O)),
                    )
                # prewarm the Sqrt ACT table for the next squash
                nc.scalar.activation(
                    scr[:, 0:1], eb[:, K * U - 1:K * U],
                    mybir.ActivationFunctionType.Sqrt,
                )
                weff = weff_s

    nc.compile()
    return nc


_PROGRAM_CACHE = {}


def _get_program():
    if "nc" not in _PROGRAM_CACHE:
        _PROGRAM_CACHE["nc"] = _build_program()
    return _PROGRAM_CACHE["nc"]


def _make_in_maps(x, W):
    x = np.ascontiguousarray(x, dtype=np.float32)
    W = np.ascontiguousarray(W, dtype=np.float32)
    bf16 = ml_dtypes.bfloat16
    sel2 = np.zeros((128, 128), dtype=np.float32)
    for p in range(128):
        g = p // IU
        sel2[p, g * IU:(g + 1) * IU] = 1.0 / B
    sel2 = sel2.astype(bf16)

    in_maps = []
    for core in range(N_CORES):
        c0 = core * CL
        xc = x[:, :, c0:c0 + CL]                    # [B, I, CL]
        Wc = W[c0:c0 + CL]                          # [CL, U, O, I]
        # xp[p, k*B + b] = x[b, i, c], ci = k*128+p = c_rel*8+i
        xp = xc.transpose(2, 1, 0).reshape(CI, B)
        xp = np.ascontiguousarray(
            xp.reshape(K, 128, B).transpose(1, 0, 2).reshape(128, K * B)
        ).astype(bf16)
        # xb[p, t*CI + ci] = x[t*128+p, i, c]
        xb = xc.transpose(0, 2, 1).reshape(B, CI)
        xb = np.ascontiguousarray(
            xb.reshape(2, 128, CI).transpose(1, 0, 2).reshape(128, 2 * CI)
        ).astype(bf16)
        # w1[p, k*UO + uo] = W[c, u, o, i]
        w1 = Wc.transpose(0, 3, 1, 2).reshape(CI, UO)
        w1 = np.ascontiguousarray(
            w1.reshape(K, 128, UO).transpose(1, 0, 2).reshape(128, K * UO)
        ).astype(bf16)
        in_maps.append({"xp": xp, "xb": xb, "w1": w1, "sel2": sel2})
    return in_maps


def kernel(x, W, _trace=False, _trace_kwargs=None):
    nc = _get_program()
    in_maps = _make_in_maps(x, W)
    res = run_bass_kernel_spmd(
        nc, in_maps, core_ids=list(range(N_CORES)), trace=_trace,
        **(_trace_kwargs or {}),
    )
    out = np.concatenate(
        [res.results[q]["out"].astype(np.float32) for q in range(N_CORES)], axis=0
    ).reshape(B, U, O, 1)
    if _trace:
        kernel.last_results = res
    return out
